# revision 1
# baseline (speedup 1.0000x reference)
"""Sparse cross-attention (squared-ReLU normalizer) on 8 TRN2 NeuronCores.

Sharding: 8 cores = batch(2) x head-group(4). Each core owns one batch and
4 of 16 heads (a 256-wide slice of hsize): Wq/Wkv column-parallel,
Wo row-parallel (partial outputs summed on host), mask replicated per
batch shard.

Per-core kernel (all matmuls bf16 w/ fp32 PSUM accumulation):
  rqT (hs, q) = WqT-slice projection (1/sqrt(adim) folded in), rkT (hs, s),
  rv (s, hs + ones cols) from kT chunks.
  per (q-tile, head):
    S^T chunk (s128, q512) = rkT_h-as-weights matmul vs rqT_h   (K=64)
    r = Relu(S^T + nbias)            [ACT evict]
    t = (r * maskT) * r              [DVE bf16]
    o (q128, 64) + denom col = t^T-chunk-as-weights matmul vs rv[,65]
    o_scaled = o * 1/(denom+eps)     [per-partition tensor_scalar]
    oT via PE transpose; out partial (q,1024) = oT^T-as-weights @ woT  [fp32]
"""

import numpy as np
import ml_dtypes

BF16 = ml_dtypes.bfloat16

B, Q, S, D = 2, 2048, 2048, 1024
NUM_HEAD, ADIM = 16, 64
HSIZE = NUM_HEAD * ADIM
N_CORES = 8
GROUPS = 4                  # head groups (tensor-parallel dim)
HPG = NUM_HEAD // GROUPS    # 4 heads per core
HS = HPG * ADIM             # 256: per-core hsize slice
IEPS = 1e-32
P = 128

_COMPILED = None


def _build(q=Q, s=S, d=D, hpg=HPG, adim=ADIM, qt=512):
    """Build + compile the per-core Bass program. Returns the Bacc."""
    from contextlib import ExitStack
    import concourse.bass as bass
    import concourse.mybir as mybir
    import concourse.tile as tile
    from concourse import bacc
    from concourse.masks import make_identity

    fp32 = mybir.dt.float32
    bf16 = mybir.dt.bfloat16

    hs = hpg * adim
    DC = d // P          # contraction chunks for projections
    NQ = q // qt         # q tiles
    SC = s // P          # s chunks
    QCT = qt // P        # q 128-chunks per q tile
    HC = hs // P         # hsize-slice chunks (2)
    ND = d // 512        # output d tiles
    assert hs % P == 0 and q % qt == 0 and qt % P == 0 and d % 512 == 0

    nc = bacc.Bacc("TRN2", target_bir_lowering=False, debug=False,
                   num_devices=N_CORES)

    qT = nc.dram_tensor("qT", [d, q], bf16, kind="ExternalInput").ap()
    kT = nc.dram_tensor("kT", [d, s], bf16, kind="ExternalInput").ap()
    wqT = nc.dram_tensor("wqT", [d, hs], bf16, kind="ExternalInput").ap()
    wkT = nc.dram_tensor("wkT", [d, hs], bf16, kind="ExternalInput").ap()
    wvT = nc.dram_tensor("wvT", [d, hs], bf16, kind="ExternalInput").ap()
    woT = nc.dram_tensor("woT", [hs, d], bf16, kind="ExternalInput").ap()
    maskT = nc.dram_tensor("maskT", [s, q], bf16, kind="ExternalInput").ap()
    nbias = nc.dram_tensor("nbias", [1, 1], fp32, kind="ExternalInput").ap()
    out = nc.dram_tensor("out", [q, d], fp32, kind="ExternalOutput").ap()

    qT_t = qT.rearrange("(c p) q -> c p q", p=P)      # [DC, 128, q]
    kT_t = kT.rearrange("(c p) s -> c p s", p=P)
    wqT_t = wqT.rearrange("(c p) h -> c p h", p=P)
    wkT_t = wkT.rearrange("(c p) h -> c p h", p=P)
    wvT_t = wvT.rearrange("(c p) h -> c p h", p=P)
    woT_t = woT.rearrange("(c p) d -> c p d", p=P)    # [HC, 128, d]
    maskT_t = maskT.rearrange("(c p) q -> p c q", p=P)  # [128, SC, q]
    out_t = out.rearrange("(c p) d -> c p d", p=P)    # [q/P, 128, d]

    with tile.TileContext(nc) as tc, ExitStack() as ctx:
        const = ctx.enter_context(tc.tile_pool(name="const", bufs=1))
        wpool = ctx.enter_context(tc.tile_pool(name="w", bufs=1))
        xpool = ctx.enter_context(tc.tile_pool(name="x", bufs=10))
        actp = ctx.enter_context(tc.tile_pool(name="act", bufs=1))
        tTp = ctx.enter_context(tc.tile_pool(name="tT", bufs=2))
        maskp = ctx.enter_context(tc.tile_pool(name="mask", bufs=2))
        rp = ctx.enter_context(tc.tile_pool(name="r", bufs=4))
        smallp = ctx.enter_context(tc.tile_pool(name="small", bufs=8))
        outp = ctx.enter_context(tc.tile_pool(name="out", bufs=3))
        oblkp = ctx.enter_context(tc.tile_pool(name="oblk", bufs=8))
        psA = ctx.enter_context(tc.tile_pool(name="psA", bufs=2, space="PSUM"))
        psS = ctx.enter_context(tc.tile_pool(name="psS", bufs=3, space="PSUM"))
        psO = ctx.enter_context(tc.tile_pool(name="psO", bufs=2, space="PSUM"))
        psT = ctx.enter_context(tc.tile_pool(name="psT", bufs=1, space="PSUM"))

        # ---- constants ----
        ident = const.tile([P, P], bf16)
        make_identity(nc, ident)
        ones1 = const.tile([1, P], fp32)
        nc.any.memset(ones1[:], 1.0)
        nb1 = const.tile([1, 1], fp32)
        nc.sync.dma_start(nb1[:], nbias[:])
        # broadcast nbias to all 128 partitions via K=1 matmul outer product
        ps_nb = psT.tile([P, P], fp32, tag="pt")
        nc.tensor.matmul(ps_nb[:, 0:1], ones1[:], nb1[:], start=True, stop=True)
        nb128 = const.tile([P, 1], fp32)
        nc.scalar.copy(nb128[:], ps_nb[:, 0:1])

        # ---- resident weights ----
        wq_sb = wpool.tile([P, DC, hs], bf16)
        wk_sb = wpool.tile([P, DC, hs], bf16)
        wv_sb = wpool.tile([P, DC, hs], bf16)
        wo_sb = wpool.tile([P, HC, d], bf16)
        for c in range(DC):
            nc.sync.dma_start(wq_sb[:, c], wqT_t[c])
            nc.sync.dma_start(wk_sb[:, c], wkT_t[c])
            nc.sync.dma_start(wv_sb[:, c], wvT_t[c])
        for c in range(HC):
            nc.sync.dma_start(wo_sb[:, c], woT_t[c])

        # ---- activations (resident) ----
        rqT_sb = actp.tile([P, HC, q], bf16)    # (hs, q)
        rkT_sb = actp.tile([P, HC, s], bf16)    # (hs, s)
        rv_sb = actp.tile([P, SC, hpg * (adim + 1)], bf16)  # (s, hs + ones)
        oT_sb = actp.tile([P, HC, q], bf16)     # (hs, q)
        nc.any.memset(rv_sb[:], 1.0)            # ones cols survive at 64::65

        scale = 1.0 / np.sqrt(np.float32(adim))

        # ---- stage A1: rqT = (Wq_slice @ iQ[b]^T) scaled ----
        x_tiles = []
        for c in range(DC):
            xt = xpool.tile([P, q], bf16, tag="xch")
            nc.sync.dma_start(xt[:], qT_t[c])
            x_tiles.append(xt)
        for m in range(HC):
            for nq in range(q // 512):
                ps = psA.tile([P, 512], fp32)
                for c in range(DC):
                    nc.tensor.matmul(
                        ps[:], wq_sb[:, c, m * P:(m + 1) * P],
                        x_tiles[c][:, nq * 512:(nq + 1) * 512],
                        start=(c == 0), stop=(c == DC - 1))
                # fold the 1/sqrt(adim) score scale into rq
                nc.scalar.activation(
                    rqT_sb[:, m, nq * 512:(nq + 1) * 512], ps[:],
                    mybir.ActivationFunctionType.Copy, scale=float(scale))

        # ---- stage A2: rkT and rv (x tiles re-used for kT) ----
        x_tiles = []
        for c in range(DC):
            xt = xpool.tile([P, s], bf16, tag="xch")
            nc.sync.dma_start(xt[:], kT_t[c])
            x_tiles.append(xt)
        for m in range(HC):
            for nq in range(s // 512):
                ps = psA.tile([P, 512], fp32)
                for c in range(DC):
                    nc.tensor.matmul(
                        ps[:], wk_sb[:, c, m * P:(m + 1) * P],
                        x_tiles[c][:, nq * 512:(nq + 1) * 512],
                        start=(c == 0), stop=(c == DC - 1))
                nc.scalar.copy(rkT_sb[:, m, nq * 512:(nq + 1) * 512], ps[:])
        for sc in range(SC):
            ps = psA.tile([P, 512], fp32)
            for c in range(DC):
                nc.tensor.matmul(
                    ps[:, :hs], x_tiles[c][:, sc * P:(sc + 1) * P],
                    wv_sb[:, c], start=(c == 0), stop=(c == DC - 1))
            # scatter heads into 65-strided groups (col 64 of each stays 1.0)
            nc.scalar.copy(
                rv_sb[:, sc].rearrange("p (h c) -> p h c", c=adim + 1)[:, :, 0:adim],
                ps[:, :hs].rearrange("p (h c) -> p h c", c=adim))

        # ---- stage B + C per q tile ----
        for iq in range(NQ):
            qlo = iq * qt
            mblk = maskp.tile([P, SC, qt], bf16)
            nc.sync.dma_start(mblk[:], maskT_t[:, :, qlo:qlo + qt])
            o_blks = [oblkp.tile([P, hs], bf16, tag="oblk", name=f"oblk{iq}_{i}")
                      for i in range(QCT)]
            tTs = {}

            def scores_block(h, iq=iq, qlo=qlo, mblk=mblk, tTs=tTs):
                hp = (h % 2) * adim
                hc = h // 2
                tT = tTp.tile([P, SC, qt], bf16, tag="tT", name=f"tT{iq}_{h}")
                tTs[h] = tT
                for sc in range(SC):
                    ps = psS.tile([P, qt], fp32, name="ps_s")
                    nc.tensor.matmul(
                        ps[:], rkT_sb[hp:hp + adim, hc, sc * P:(sc + 1) * P],
                        rqT_sb[hp:hp + adim, hc, qlo:qlo + qt],
                        start=True, stop=True)
                    r = rp.tile([P, qt], bf16, tag="r", name="r_t")
                    nc.scalar.activation(
                        r[:], ps[:], mybir.ActivationFunctionType.Relu,
                        bias=nb128[:])
                    rm = rp.tile([P, qt], bf16, tag="rm", name="rm_t")
                    nc.vector.tensor_mul(rm[:], r[:], mblk[:, sc])
                    nc.vector.tensor_mul(tT[:, sc], rm[:], r[:])

            def av_block(h, o_blks=o_blks, tTs=tTs):
                tT = tTs.pop(h)
                for qc in range(QCT):
                    po = psO.tile([P, adim + 1], fp32, tag="po", name="po_t")
                    for sc in range(SC):
                        nc.tensor.matmul(
                            po[:], tT[:, sc, qc * P:(qc + 1) * P],
                            rv_sb[:, sc, h * (adim + 1):(h + 1) * (adim + 1)],
                            start=(sc == 0), stop=(sc == SC - 1))
                    den = smallp.tile([P, 1], fp32, tag="den", name="den_t")
                    nc.vector.tensor_scalar_add(den[:], po[:, adim:adim + 1],
                                                IEPS)
                    rec = smallp.tile([P, 1], fp32, tag="rec", name="rec_t")
                    nc.vector.reciprocal(rec[:], den[:])
                    nc.vector.tensor_scalar_mul(
                        o_blks[qc][:, h * adim:(h + 1) * adim],
                        po[:, 0:adim], rec[:])

            for h in range(hpg):
                scores_block(h)
                av_block(h)

            # transpose o (q,hs) -> oT (hs,q), then out partial = oT^T @ woT
            for qc in range(QCT):
                for c in range(HC):
                    pt = psT.tile([P, P], bf16, tag="pt", name="pt_t")
                    nc.tensor.transpose(
                        pt[:], o_blks[qc][:, c * P:(c + 1) * P], ident[:])
                    nc.scalar.copy(
                        oT_sb[:, c, qlo + qc * P:qlo + (qc + 1) * P], pt[:])
            for qc in range(QCT):
                for nd in range(ND):
                    ps = psA.tile([P, 512], fp32, tag="ps", name="ps_c")
                    for c in range(HC):
                        nc.tensor.matmul(
                            ps[:], oT_sb[:, c, qlo + qc * P:qlo + (qc + 1) * P],
                            wo_sb[:, c, nd * 512:(nd + 1) * 512],
                            start=(c == 0), stop=(c == HC - 1))
                    ob = outp.tile([P, 512], fp32, tag="ob", name="ob_t")
                    nc.scalar.copy(ob[:], ps[:])
                    nc.sync.dma_start(
                        out_t[iq * QCT + qc, :, nd * 512:(nd + 1) * 512], ob[:])

    nc.compile()
    return nc


def _shard_inputs(iQ, iK, mask, Wq, Wkv, Wo, nbias):
    in_maps = []
    maskT_by_b = [np.ascontiguousarray((~mask[b]).T).astype(BF16)
                  for b in range(B)]
    qT_by_b = [np.ascontiguousarray(iQ[b].T).astype(BF16) for b in range(B)]
    kT_by_b = [np.ascontiguousarray(iK[b].T).astype(BF16) for b in range(B)]
    nb = np.asarray(nbias, np.float32).reshape(1, 1)
    for ci in range(N_CORES):
        b, g = ci // GROUPS, ci % GROUPS
        hsl = slice(g * HS, (g + 1) * HS)
        in_maps.append({
            "qT": qT_by_b[b],
            "kT": kT_by_b[b],
            "wqT": np.ascontiguousarray(Wq[hsl].T).astype(BF16),
            "wkT": np.ascontiguousarray(Wkv[hsl].T).astype(BF16),
            "wvT": np.ascontiguousarray(Wkv[HSIZE + g * HS:HSIZE + (g + 1) * HS].T).astype(BF16),
            "woT": np.ascontiguousarray(Wo[:, hsl].T).astype(BF16),
            "maskT": maskT_by_b[b],
            "nbias": nb,
        })
    return in_maps


def kernel(iQ, iK, mask, Wq, Wkv, Wo, nbias):
    global _COMPILED
    from concourse.bass_utils import run_bass_kernel_spmd

    if _COMPILED is None:
        _COMPILED = _build()
    in_maps = _shard_inputs(np.asarray(iQ, np.float32), np.asarray(iK, np.float32),
                            np.asarray(mask), np.asarray(Wq, np.float32),
                            np.asarray(Wkv, np.float32), np.asarray(Wo, np.float32),
                            np.asarray(nbias, np.float32))
    res = run_bass_kernel_spmd(_COMPILED, in_maps, list(range(N_CORES))).results
    out = np.zeros((B, Q, D), np.float32)
    for ci in range(N_CORES):
        out[ci // GROUPS] += np.asarray(res[ci]["out"], np.float32)
    return out



# revision 25
# speedup vs baseline: 1.0956x; 1.0956x over previous
"""Sparse cross-attention (squared-ReLU normalizer) on 8 TRN2 NeuronCores.

Sharding: 8 cores = batch(2) x head-group(4). Each core owns one batch and
4 of 16 heads (a 256-wide slice of hsize): Wq/Wkv column-parallel,
Wo row-parallel (partial outputs summed on host), mask replicated per
batch shard.

v2 design (vs v1 baseline):
  - AV matmul restructured: rv-as-stationary [128(s),65], streaming tT
    N=512 -> oT directly in PSUM (no 128x128 ldweights per 65-col stream,
    no PE transposes). Odd heads write at psum partition offset 63 with a
    leading ones-column so all lanes stay aligned with the packed oT/wo
    layout (engines cannot shift partitions).
  - Denominator: ones col in rv -> den row in psum; DVE reciprocal ->
    gpsimd partition_broadcast -> one DVE multiply scales oT.
  - Score elementwise is 2-3 ops split across ACT/DVE/Pool:
      A-route: r = Relu(ps) [ACT evict]; u1 = r*m [DVE]; t = (u1 max 0)*u1
      B-route: w = ps*m [DVE evict]; t = (w max 0)*w  [STT on DVE or Pool]
    (nbias==0 fast path; nbias!=0 falls back to all-A with ACT bias.)
  - Stage A projections run contraction-outer across 8 PSUM banks so the
    PE starts as soon as the first qT/kT chunk lands (p-state ramp).
  - PE instruction stream is software-pipelined: S(u+1) issues before
    AV(u) so the tensor engine never waits on the elementwise chain.
  - Output partials DMA'd to HBM straight from PSUM.
"""

import numpy as np
import ml_dtypes

BF16 = ml_dtypes.bfloat16

B, Q, S, D = 2, 2048, 2048, 1024
NUM_HEAD, ADIM = 16, 64
HSIZE = NUM_HEAD * ADIM
N_CORES = 8
GROUPS = 4                  # head groups (tensor-parallel dim)
HPG = NUM_HEAD // GROUPS    # 4 heads per core
HS = HPG * ADIM             # 256: per-core hsize slice
P = 128
QT = 512

# elementwise routing (tunable): score s-chunks come in 4 pairs of 4 chunks
SQ_POOL_PAIRS = (0, 1)      # pairs whose square runs on gpsimd (Pool)

_COMPILED = None
_COMPILED_BY = {}
_LAST_NB_ZERO = True
DEBUG_TAPS = False


def _build(nb_zero=True, q=Q, s=S, d=D, hpg=HPG, adim=ADIM, qt=QT):
    """Build + compile the per-core Bass program. Returns the Bacc."""
    from contextlib import ExitStack
    import concourse.bass as bass
    import concourse.mybir as mybir
    import concourse.tile as tile
    from concourse import bacc

    fp32 = mybir.dt.float32
    bf16 = mybir.dt.bfloat16
    AF = mybir.ActivationFunctionType
    ALU = mybir.AluOpType

    hs = hpg * adim
    DC = d // P          # contraction chunks for projections (8)
    NQ = q // qt         # q tiles (4)
    SC = s // P          # s chunks (16)
    HC = hs // P         # hsize-slice chunks (2)
    G65 = adim + 1       # rv group width (64 data + ones col)
    assert hs % P == 0 and q % qt == 0 and d % 512 == 0

    nc = bacc.Bacc("TRN2", target_bir_lowering=False, debug=False,
                   num_devices=N_CORES)

    qT = nc.dram_tensor("qT", [d, q], bf16, kind="ExternalInput").ap()
    kT = nc.dram_tensor("kT", [d, s], bf16, kind="ExternalInput").ap()
    wqT = nc.dram_tensor("wqT", [d, hs], bf16, kind="ExternalInput").ap()
    wkT = nc.dram_tensor("wkT", [d, hs], bf16, kind="ExternalInput").ap()
    wvT = nc.dram_tensor("wvT", [d, hs], bf16, kind="ExternalInput").ap()
    woT = nc.dram_tensor("woT", [hs, d], bf16, kind="ExternalInput").ap()
    maskT = nc.dram_tensor("maskT", [s, q], bf16, kind="ExternalInput").ap()
    if not nb_zero:
        nbias = nc.dram_tensor("nbias", [1, 1], fp32, kind="ExternalInput").ap()
    out = nc.dram_tensor("out", [q, d], bf16, kind="ExternalOutput").ap()
    if DEBUG_TAPS:
        dbg_rq = nc.dram_tensor("dbg_rq", [P, 2 * q], bf16, kind="ExternalOutput").ap()
        dbg_rk = nc.dram_tensor("dbg_rk", [P, 2 * s], bf16, kind="ExternalOutput").ap()
        dbg_rv = nc.dram_tensor("dbg_rv", [P, (s // P) * hpg * (adim + 1)], bf16, kind="ExternalOutput").ap()
        dbg_oT = nc.dram_tensor("dbg_oT", [P, 2 * q], bf16, kind="ExternalOutput").ap()

    qT_t = qT.rearrange("(c p) q -> c p q", p=P)        # [DC, 128, q]
    kT_t = kT.rearrange("(c p) s -> c p s", p=P)
    wqT_t = wqT.rearrange("(c p) h -> p c h", p=P)      # [128, DC, hs]
    wkT_t = wkT.rearrange("(c p) h -> p c h", p=P)
    wvT_t = wvT.rearrange("(c p) h -> p c h", p=P)
    woT_t = woT.rearrange("(c p) d -> p c d", p=P)      # [128, HC, d]
    maskT_t = maskT.rearrange("(c p) q -> p c q", p=P)  # [128, SC, q]
    out_t = out.rearrange("(t p) (n e) -> t p n e", p=P, n=2)  # [q/P,128,2,512]

    with tile.TileContext(nc) as tc, ExitStack() as ctx:
        wpool = ctx.enter_context(tc.tile_pool(name="w", bufs=1))
        xpool = ctx.enter_context(tc.tile_pool(name="x", bufs=9))
        actp = ctx.enter_context(tc.tile_pool(name="act", bufs=1))
        tTp = ctx.enter_context(tc.tile_pool(name="tT", bufs=2))
        maskp = ctx.enter_context(tc.tile_pool(name="mask", bufs=2))
        rp = ctx.enter_context(tc.tile_pool(name="r", bufs=4))
        smallp = ctx.enter_context(tc.tile_pool(name="small", bufs=3))
        outp = ctx.enter_context(tc.tile_pool(name="out", bufs=3))

        scale = 1.0 / np.sqrt(np.float32(adim))

        # ---- resident weights (one batched DMA each) ----
        wq_sb = wpool.tile([P, DC, hs], bf16)
        wk_sb = wpool.tile([P, DC, hs], bf16)
        wv_sb = wpool.tile([P, DC, hs], bf16)
        wo_sb = wpool.tile([P, HC, d], bf16)

        # ---- activations (resident) ----
        rqT_sb = actp.tile([P, HC, q], bf16)    # (hs, q), scale folded in
        rkT_sb = actp.tile([P, HC, s], bf16)    # (hs, s)
        rv_sb = actp.tile([P, SC, hpg * G65], bf16)  # (s, hs + ones cols)
        oT_sb = actp.tile([P, HC, q], bf16)     # (hs, q), scaled
        nc.any.memset(rv_sb[:], 1.0)            # ones cols survive

        if not nb_zero:
            nb1 = smallp.tile([1, 1], fp32, tag="nb1")
            nc.sync.dma_start(nb1[:], nbias[:])
            nb128 = smallp.tile([P, 1], fp32, tag="nb128")
            nc.gpsimd.partition_broadcast(nb128[:], nb1[:], channels=P)

        # ---- input DMAs, ordered by first use ----
        nc.sync.dma_start(wq_sb[:], wqT_t[:])
        xq = []
        for c in range(DC):
            xt = xpool.tile([P, q], bf16, tag="xch", name=f"xq{c}")
            nc.sync.dma_start(xt[:], qT_t[c])
            xq.append(xt)
        nc.sync.dma_start(wk_sb[:], wkT_t[:])
        nc.sync.dma_start(wv_sb[:], wvT_t[:])
        xk = []
        for c in range(DC):
            xt = xpool.tile([P, s], bf16, tag="xch", name=f"xk{c}")
            nc.sync.dma_start(xt[:], kT_t[c])
            xk.append(xt)
        nc.sync.dma_start(wo_sb[:], woT_t[:])

        mblks = {}

        def mask_dma(iq):
            mb = maskp.tile([P, SC, qt], bf16, tag="m", name=f"m{iq}")
            nc.sync.dma_start(mb[:], maskT_t[:, :, iq * qt:(iq + 1) * qt])
            mblks[iq] = mb

        mask_dma(0)

        # ---- stage A: projections, contraction-outer over 8 PSUM banks ----
        with tc.tile_pool(name="psA", bufs=8, space="PSUM") as psA:
            # A1: rqT = scale * (Wq_slice @ iQ^T)
            psq = [psA.tile([P, qt], fp32, tag="psa", name=f"psq{j}") for j in range(8)]
            for c in range(DC):
                for m in range(HC):
                    for nq in range(NQ):
                        nc.tensor.matmul(
                            psq[m * NQ + nq][:],
                            wq_sb[:, c, m * P:(m + 1) * P],
                            xq[c][:, nq * qt:(nq + 1) * qt],
                            start=(c == 0), stop=(c == DC - 1))
            for m in range(HC):
                for nq in range(NQ):
                    nc.scalar.activation(
                        rqT_sb[:, m, nq * qt:(nq + 1) * qt],
                        psq[m * NQ + nq][:], AF.Copy, scale=float(scale))
            # A2a: rkT
            psk = [psA.tile([P, qt], fp32, tag="psa", name=f"psk{j}") for j in range(8)]
            for c in range(DC):
                for m in range(HC):
                    for nq in range(NQ):
                        nc.tensor.matmul(
                            psk[m * NQ + nq][:],
                            wk_sb[:, c, m * P:(m + 1) * P],
                            xk[c][:, nq * qt:(nq + 1) * qt],
                            start=(c == 0), stop=(c == DC - 1))
            for m in range(HC):
                for nq in range(NQ):
                    nc.scalar.activation(
                        rkT_sb[:, m, nq * qt:(nq + 1) * qt],
                        psk[m * NQ + nq][:], AF.Copy)
            # A2b: rv (x chunks as stationary), two passes of 8 s-chunks
            for half in range(2):
                psv = [psA.tile([P, hs], fp32, tag="psa", name=f"psv{half}_{j}")
                       for j in range(8)]
                for c in range(DC):
                    for j in range(8):
                        sc = half * 8 + j
                        nc.tensor.matmul(
                            psv[j][:],
                            xk[c][:, sc * P:(sc + 1) * P],
                            wv_sb[:, c, :],
                            start=(c == 0), stop=(c == DC - 1))
                for j in range(8):
                    sc = half * 8 + j
                    src = psv[j][:].rearrange("p (h c) -> p h c", h=hpg)
                    dst = rv_sb[:, sc].rearrange("p (h c) -> p h c", c=G65)
                    # data at cols 0..63 of each 65-group; ones col at 64
                    nc.scalar.activation(dst[:, :, 0:adim], src[:], AF.Copy)

        # ---- main loop ----
        psS = ctx.enter_context(tc.tile_pool(name="psS", bufs=3, space="PSUM"))
        psO = ctx.enter_context(tc.tile_pool(name="psO", bufs=2, space="PSUM"))

        # odd heads first within each q-tile: their scaled output reaches the
        # packed oT tile via a partition-shifting SBUF DMA, which this order
        # hides behind the following units before outproj needs it
        units = [(iq, h) for iq in range(NQ) for h in (1, 3, 0, 2)]
        tTs = {}

        ones64 = smallp.tile([1, adim], bf16, tag="ones64")
        nc.any.memset(ones64[:], 1.0)

        def scores_block(u):
            iq, h = units[u]
            hc, hp = h // 2, (h % 2) * adim
            qlo = iq * qt
            if h == 0 and iq + 1 < NQ and iq + 1 not in mblks:
                mask_dma(iq + 1)
            mblk = mblks[iq]
            tT = tTp.tile([P, SC, qt], bf16, tag="tT", name=f"tT{u}")
            tTs[u] = tT
            for pair in range(4):
                r = rp.tile([P, 4, qt], bf16, tag="rw", name=f"r{u}_{pair}")
                for gg in range(2):
                    g = pair * 2 + gg
                    ps = psS.tile([P, 2, qt], fp32, tag="ps", name=f"s{u}_{g}")
                    for k in range(2):
                        sc = 2 * g + k
                        nc.tensor.matmul(
                            ps[:, k],
                            rkT_sb[hp:hp + adim, hc, sc * P:(sc + 1) * P],
                            rqT_sb[hp:hp + adim, hc, qlo:qlo + qt],
                            start=True, stop=True)
                    if nb_zero:
                        nc.scalar.activation(r[:, 2 * gg:2 * gg + 2], ps[:],
                                             AF.Relu)
                    else:
                        nc.scalar.activation(r[:, 2 * gg:2 * gg + 2], ps[:],
                                             AF.Relu, bias=nb128[:])
                u1 = rp.tile([P, 4, qt], bf16, tag="rw", name=f"u{u}_{pair}")
                nc.vector.tensor_mul(u1[:], r[:], mblk[:, 4 * pair:4 * pair + 4])
                tv = tT[:, 4 * pair:4 * pair + 4]
                sq_eng = nc.gpsimd if pair in SQ_POOL_PAIRS else nc.vector
                sq_eng.tensor_mul(tv, u1[:], u1[:])

        def av_block(u):
            iq, h = units[u]
            hc, odd = h // 2, h % 2
            qlo = iq * qt
            tT = tTs.pop(u)
            pso = psO.tile([P, qt], fp32, tag="po", name=f"po{u}")
            for sc in range(SC):
                nc.tensor.matmul(
                    pso[0:G65, :],
                    rv_sb[:, sc, h * G65:(h + 1) * G65],
                    tT[:, sc], start=(sc == 0), stop=(sc == SC - 1))
            denb = smallp.tile([1, qt], bf16, tag="denb", name=f"denb{u}")
            with nc.allow_low_precision(reason="attn denominator broadcast"):
                nc.scalar.activation(denb[:], pso[adim:adim + 1, :], AF.Copy)
            # broadcast den across partitions via K=1 ones outer product,
            # then a fast approximate reciprocal on all 64 lanes at once
            pb = psO.tile([P, qt], fp32, tag="po", name=f"pb{u}")
            nc.tensor.matmul(pb[0:adim, :], ones64[:], denb[:],
                             start=True, stop=True)
            recB = smallp.tile([adim, qt], fp32, tag="recB", name=f"recB{u}")
            nc.vector.reciprocal_approx_fast(recB[:], pb[0:adim, :])
            if odd:
                ost = smallp.tile([adim, qt], bf16, tag="ost", name=f"ost{u}")
                nc.vector.tensor_mul(ost[:], pso[0:adim, :], recB[:])
                nc.sync.dma_start(oT_sb[adim:P, hc, qlo:qlo + qt], ost[:])
            else:
                nc.vector.tensor_mul(
                    oT_sb[0:adim, hc, qlo:qlo + qt], pso[0:adim, :], recB[:])

        def outproj(iq):
            qlo = iq * qt
            for qc in range(NQ):
                pso = psS.tile([P, 2, 512], fp32, tag="ps", name=f"o{iq}_{qc}")
                for nd in range(2):
                    for c in range(HC):
                        nc.tensor.matmul(
                            pso[:, nd],
                            oT_sb[:, c, qlo + qc * P:qlo + (qc + 1) * P],
                            wo_sb[:, c, nd * 512:(nd + 1) * 512],
                            start=(c == 0), stop=(c == HC - 1))
                ob = outp.tile([P, 2, 512], bf16, tag="ob", name=f"ob{iq}_{qc}")
                if qc % 2 == 0:
                    nc.scalar.activation(ob[:], pso[:], AF.Copy)
                else:
                    nc.vector.tensor_copy(ob[:], pso[:])
                nc.sync.dma_start(out_t[iq * NQ + qc], ob[:])

        scores_block(0)
        for u in range(len(units)):
            if u + 1 < len(units):
                scores_block(u + 1)
            av_block(u)
            if (u + 1) % hpg == 0:          # last head of this q-tile done
                outproj(units[u][0])

        if DEBUG_TAPS:
            nc.sync.dma_start(dbg_rq.rearrange("p (c q) -> p c q", c=2), rqT_sb[:])
            nc.sync.dma_start(dbg_rk.rearrange("p (c q) -> p c q", c=2), rkT_sb[:])
            nc.sync.dma_start(dbg_rv.rearrange("p (c g) -> p c g", c=SC), rv_sb[:])
            nc.sync.dma_start(dbg_oT.rearrange("p (c q) -> p c q", c=2), oT_sb[:])

    nc.compile()
    return nc


def _shard_inputs(iQ, iK, mask, Wq, Wkv, Wo, nbias):
    in_maps = []
    maskT_by_b = [np.ascontiguousarray((~mask[b]).T).astype(BF16)
                  for b in range(B)]
    qT_by_b = [np.ascontiguousarray(iQ[b].T).astype(BF16) for b in range(B)]
    kT_by_b = [np.ascontiguousarray(iK[b].T).astype(BF16) for b in range(B)]
    nb = np.asarray(nbias, np.float32).reshape(1, 1)
    for ci in range(N_CORES):
        b, g = ci // GROUPS, ci % GROUPS
        hsl = slice(g * HS, (g + 1) * HS)
        m = {
            "qT": qT_by_b[b],
            "kT": kT_by_b[b],
            "wqT": np.ascontiguousarray(Wq[hsl].T).astype(BF16),
            "wkT": np.ascontiguousarray(Wkv[hsl].T).astype(BF16),
            "wvT": np.ascontiguousarray(Wkv[HSIZE + g * HS:HSIZE + (g + 1) * HS].T).astype(BF16),
            "woT": np.ascontiguousarray(Wo[:, hsl].T).astype(BF16),
            "maskT": maskT_by_b[b],
        }
        if not _LAST_NB_ZERO:
            m["nbias"] = nb
        in_maps.append(m)
    return in_maps


def kernel(iQ, iK, mask, Wq, Wkv, Wo, nbias):
    global _COMPILED, _LAST_NB_ZERO
    from concourse.bass_utils import run_bass_kernel_spmd

    nbv = float(np.asarray(nbias, np.float32).reshape(-1)[0])
    nb_zero = (nbv == 0.0)
    _LAST_NB_ZERO = nb_zero
    if nb_zero not in _COMPILED_BY:
        _COMPILED_BY[nb_zero] = _build(nb_zero=nb_zero)
    _COMPILED = _COMPILED_BY[nb_zero]

    in_maps = _shard_inputs(np.asarray(iQ, np.float32), np.asarray(iK, np.float32),
                            np.asarray(mask), np.asarray(Wq, np.float32),
                            np.asarray(Wkv, np.float32), np.asarray(Wo, np.float32),
                            np.asarray(nbias, np.float32))
    res = run_bass_kernel_spmd(_COMPILED, in_maps, list(range(N_CORES))).results
    out = np.zeros((B, Q, D), np.float32)
    for ci in range(N_CORES):
        out[ci // GROUPS] += np.asarray(res[ci]["out"], np.float32)
    return out
